# revision 18
# baseline (speedup 1.0000x reference)
"""Trainium2 Bass kernel for CombinedLora (moe_routing).

Contract: kernel(**inputs) takes FULL inputs (lora_A [128,4096,64] f16,
lora_B [128,64,4096] f16, x [256,1,4096] f16, xids [10240] i32,
wids [160] i32) and returns the FULL output [256,1,4096] f16.

Strategy (fused single launch, expert-parallel over 8 cores):
  reference:
    lv[c,r]   = sum_k x[xids[c*64+r],k] * lora_A[wids[c],k,r]      (C=160 rows)
    out[t,:]  = SCALE * sum_{c,r: xids[c*64+r]=t} lv[wids[c],r] * lora_B[wids[c],r,:]
  Only lv rows w in W = unique(wids) are consumed (lv is re-indexed by wids).

  Each core owns nw_pc = ceil(|W|/8) (rounded even) lv rows w and runs BOTH
  stages locally over the full hidden dim, so no cross-core traffic at all:
    stage 1 (PE): xa[j, t] = sum_k A[k, j] * x[t, k]   for its NR = nw_pc*64
      j-slots (j=(w,r), A column = lora_A[wids[w]][:, r]); lv[j] is then
      extracted with a host-baked one-hot mask: lv[j] = sum_t sel[j,t]*xa[j,t]
      (DVE mult + ACT free-dim accumulate). Shipping x^T (2MB, replicated) +
      native-layout A beats shipping host-gathered Xg + transposed At.
    stage 2 (PE): partial[t, d] = sum_j mt[j,t]*lv[j] * B[j, d], a dense
      matmul over the core's own j-slots with the host-built count matrix
      mt (exact small ints, shipped fp8); psum f32, scaled by SCALE on ACT.
  Host sums the 8 partial [256, 4096] outputs (f32) - a ~2MB/core relay that
  costs no device time, far cheaper than any on-device collective here.
"""

import numpy as np


def _ensure_axon_hooks():
    """run_bass_kernel_spmd(trace=True) imports antenv.axon_hooks, which some
    images lack. Register a working NTFF hook (or a None fallback) so tracing
    works when possible and degrades gracefully otherwise."""
    import sys
    import types

    try:
        import antenv.axon_hooks  # noqa: F401
        return
    except ImportError:
        pass
    hook = None
    try:
        import contextlib
        import ctypes

        lib = ctypes.CDLL("/opt/axon/libaxon_pjrt.so")
        if hasattr(lib, "axon_start_nrt_profile"):
            lib.axon_start_nrt_profile.argtypes = [
                ctypes.POINTER(ctypes.c_int64), ctypes.c_size_t]
            lib.axon_start_nrt_profile.restype = ctypes.c_int64
            lib.axon_stop_nrt_profile.argtypes = [ctypes.c_char_p]
            lib.axon_stop_nrt_profile.restype = ctypes.c_int64

            @contextlib.contextmanager
            def hook(output_dir, device_ids):
                import jax

                jax.devices()
                if device_ids:
                    ids = (ctypes.c_int64 * len(device_ids))(*device_ids)
                    rc = lib.axon_start_nrt_profile(ids, len(device_ids))
                else:
                    rc = lib.axon_start_nrt_profile(None, 0)
                if rc != 0:
                    raise RuntimeError(f"axon_start_nrt_profile rc={rc}")
                try:
                    yield
                finally:
                    lib.axon_stop_nrt_profile(str(output_dir).encode())
    except Exception:
        hook = None
    mod = types.ModuleType("antenv.axon_hooks")
    mod._hook = hook
    mod.set_axon_ntff_profile_hook = lambda h: setattr(mod, "_hook", h)
    mod.get_axon_ntff_profile_hook = lambda: mod._hook
    sys.modules["antenv.axon_hooks"] = mod
    try:
        import antenv

        antenv.axon_hooks = mod
    except ImportError:
        pass


_ensure_axon_hooks()

B, C, R, D, A = 256, 160, 64, 4096, 128
SCALE = 2.0
N_CORES = 8
KC = D // 128   # 32 contraction chunks of 128
DC = D // 512   # 8 output d-slabs of 512

_prog_cache = {}
last_results = None  # (BassKernelResults,) of the last run


def _build_fused(njb: int):
    """One launch: stage-1 xa matmul + masked lv extract + stage-2 matmul.

    njb = NR/128 j-blocks of 128 (w,r)-slots owned by this core.
    """
    import concourse.mybir as mybir
    import concourse.tile as tile
    from concourse import bacc

    f16 = mybir.dt.float16
    f32 = mybir.dt.float32
    f8 = mybir.dt.float8e4

    nc = bacc.Bacc("TRN2", target_bir_lowering=False, debug=False,
                   num_devices=N_CORES)
    # xt[p, kc, t] = x[t, kc*128+p]; replicated to all cores
    xt_d = nc.dram_tensor("xt", [128, KC, B], f16, kind="ExternalInput")
    # ar[p, jb, kc, jj]: lhsT chunks of A columns per j-slot (see host prep)
    ar_d = nc.dram_tensor("ar", [128, njb, KC, 128], f16, kind="ExternalInput")
    # b[p, dc, jb, dd] = lora_B[j-slot jb*128+p][dc*512+dd]; dc-major so each
    # 786KB dc-slab DMA has 6KB contiguous per-partition runs (not 1KB)
    b_d = nc.dram_tensor("b", [128, DC, njb, 512], f16, kind="ExternalInput")
    # sel[p, jb, t] one-hot: t == xids_r[w, r] for j-slot; fp8 exact
    sel_d = nc.dram_tensor("sel", [128, njb, B], f8, kind="ExternalInput")
    # mt[p, jb, t] = count of stage-2 contributions of j-slot to token t
    mt_d = nc.dram_tensor("mt", [128, njb, B], f8, kind="ExternalInput")
    out_d = nc.dram_tensor("out", [B, D], f16, kind="ExternalOutput")

    with tile.TileContext(nc) as tc:
        from contextlib import ExitStack

        ctx = ExitStack()
        with ctx:
            big_pool = ctx.enter_context(tc.tile_pool(name="big", bufs=1))
            msk_pool = ctx.enter_context(tc.tile_pool(name="msk", bufs=2))
            lv_pool = ctx.enter_context(tc.tile_pool(name="lv", bufs=1))
            ob_pool = ctx.enter_context(tc.tile_pool(name="ob", bufs=3))
            xa_psum = ctx.enter_context(
                tc.tile_pool(name="xaps", bufs=3, space="PSUM"))
            out_psum = ctx.enter_context(
                tc.tile_pool(name="ops", bufs=2, space="PSUM"))

            xt_t = big_pool.tile([128, KC, B], f16)
            ar_t = big_pool.tile([128, njb, KC, 128], f16)
            b_t = big_pool.tile([128, DC, njb, 512], f16)
            sel_t = big_pool.tile([128, njb, B], f8)
            mt_t = big_pool.tile([128, njb, B], f8)
            lv_f32 = lv_pool.tile([128, njb], f32)
            ms_t = big_pool.tile([128, njb, B], f16)

            # DMA order = need order: xt/ar0 halves let the first stage-1
            # matmul start after ~1.6MB, not the whole 3.4MB; sel/mt before
            # the first extract; b dc-slabs interleaved with ar jb-slabs so
            # each arrives before stage-1 (ar, ~4.3us/slab) and stage-2
            # (b, ~1.3us/slab) consume it.
            for h in range(2):
                sl = slice(h * (KC // 2), (h + 1) * (KC // 2))
                nc.sync.dma_start(xt_t[:, sl, :], xt_d[:, sl, :])
                nc.sync.dma_start(ar_t[:, 0, sl, :], ar_d[:, 0, sl, :])
            if njb > 1:
                nc.sync.dma_start(ar_t[:, 1], ar_d[:, 1])
            nc.sync.dma_start(b_t[:, 0], b_d[:, 0])
            nc.sync.dma_start(b_t[:, 1], b_d[:, 1])
            # sel/mt here: first needed by the jb0 extract (~t=18us), which
            # in turn gates xa psum-buffer recycling for jb3+
            nc.sync.dma_start(sel_t[:], sel_d[:])
            nc.sync.dma_start(mt_t[:], mt_d[:])
            for jb in range(2, njb):
                nc.sync.dma_start(ar_t[:, jb], ar_d[:, jb])
            for dc in range(2, DC):
                nc.sync.dma_start(b_t[:, dc], b_d[:, dc])

            # stage 1: per j-block, xa = A_chunk^T-contract-k with x^T,
            # then lv[j] = sum_t sel[j,t] * xa[j,t]
            for jb in range(njb):
                xa_ps = xa_psum.tile([128, B], f32)
                for kc in range(KC):
                    nc.tensor.matmul(
                        xa_ps[:], ar_t[:, jb, kc, :], xt_t[:, kc, :],
                        start=(kc == 0), stop=(kc == KC - 1))
                masked = msk_pool.tile([128, B], f32)
                nc.vector.tensor_tensor(
                    out=masked[:], in0=xa_ps[:], in1=sel_t[:, jb, :],
                    op=mybir.AluOpType.mult)
                junk = msk_pool.tile([128, B], f16)
                nc.scalar.activation(
                    junk[:], masked[:], mybir.ActivationFunctionType.Copy,
                    accum_out=lv_f32[:, jb:jb + 1])
                # ms[j, t] = mt[j, t] * lv[j], per-jb so stage 2 can start
                # right after the last extract instead of a serial chain
                nc.vector.tensor_tensor(
                    out=ms_t[:, jb, :], in0=mt_t[:, jb, :],
                    in1=lv_f32[:, jb, None].broadcast_to([128, B]),
                    op=mybir.AluOpType.mult)

            # stage 2: partial[t, dslab] = sum_jb ms[:,jb,th]^T @ b[:,jb,dslab]
            # pair two 512-wide d-slabs per output tile so the out DMA moves
            # 2KB-contiguous partition rows (half the packet count)
            for dp in range(DC // 2):
                for th in range(2):
                    ob = ob_pool.tile([128, 1024], f16)
                    for half in range(2):
                        dc = dp * 2 + half
                        ops = out_psum.tile([128, 512], f32)
                        for jb in range(njb):
                            nc.tensor.matmul(
                                ops[:],
                                ms_t[:, jb, th * 128:(th + 1) * 128],
                                b_t[:, dc, jb, :],
                                start=(jb == 0), stop=(jb == njb - 1))
                        nc.scalar.activation(
                            ob[:, half * 512:(half + 1) * 512], ops[:],
                            mybir.ActivationFunctionType.Copy,
                            scale=float(SCALE))
                    nc.sync.dma_start(
                        out_d[th * 128:(th + 1) * 128,
                              dp * 1024:(dp + 1) * 1024], ob[:])

    nc.compile()
    return nc


def _host_prep(lora_A, lora_B, x, xids, wids):
    W = np.unique(wids)
    nW = len(W)
    nw_pc = -(-nW // N_CORES)
    if nw_pc % 2:
        nw_pc += 1
    njb = nw_pc // 2          # j-blocks of 128 per core
    NR = nw_pc * 64           # j-slots per core

    x2d = np.ascontiguousarray(x[:, 0, :])
    xids_r = xids.reshape(C, R)

    # xt[p, kc, t] = x[t, kc*128+p]  (replicated)
    xt = np.ascontiguousarray(
        x2d.T.reshape(KC, 128, B).transpose(1, 0, 2))

    import concourse.mybir as mybir

    f8np = mybir.dt.np(mybir.dt.float8e4)

    # stage-2 count matrix over ALL slots, then slice per core
    slot_of = np.full(A, -1, np.int64)
    slot_of[W] = np.arange(nW)
    NKtot = N_CORES * NR
    Mt = np.zeros((NKtot, B), np.float16)
    kk = (slot_of[wids][:, None] * 64 + np.arange(R)[None, :]).ravel()
    tt = xids_r.ravel()
    np.add.at(Mt, (kk, tt), np.float16(1))

    maps = []
    for i in range(N_CORES):
        ws = W[i * nw_pc:(i + 1) * nw_pc]
        nv = len(ws)
        # ar[p, jb, kc, jj]: A column for j-slot (s, r), j = jb*128 + jj,
        # s = j//64, r = j%64, k = kc*128 + p
        Ag = np.zeros((nw_pc, D, R), np.float16)
        if nv:
            Ag[:nv] = lora_A[wids[ws]]
        ar = np.ascontiguousarray(
            Ag.reshape(njb, 2, KC, 128, R).transpose(3, 0, 2, 1, 4)
            .reshape(128, njb, KC, 128))
        # b[p, dc, jb, dd]: lora_B[j-slot jb*128+p][dc*512+dd]
        Bg = np.zeros((nw_pc, R, D), np.float16)
        if nv:
            Bg[:nv] = lora_B[ws]
        bt = np.ascontiguousarray(
            Bg.reshape(njb, 128, DC, 512).transpose(1, 2, 0, 3))
        # sel[p, jb, t] = (xids_r[w, r] == t) for j-slot jb*128+p
        sel = np.zeros((nw_pc * 64, B), f8np)
        if nv:
            jj = np.arange(nv * 64)
            sel[jj, xids_r[ws].ravel()] = np.float16(1)
        sel = np.ascontiguousarray(
            sel.reshape(njb, 128, B).transpose(1, 0, 2))
        # mt[p, jb, t]: count matrix slice for this core's slots
        mt = np.ascontiguousarray(
            Mt[i * NR:(i + 1) * NR].reshape(njb, 128, B)
            .transpose(1, 0, 2)).astype(f8np)
        maps.append({"xt": xt, "ar": ar, "b": bt, "sel": sel, "mt": mt})
    return njb, maps


def kernel(lora_A, lora_B, x, xids, wids):
    from concourse.bass_utils import run_bass_kernel_spmd

    lora_A = np.asarray(lora_A, np.float16)
    lora_B = np.asarray(lora_B, np.float16)
    x = np.asarray(x, np.float16)
    xids = np.asarray(xids, np.int32)
    wids = np.asarray(wids, np.int32)

    njb, maps = _host_prep(lora_A, lora_B, x, xids, wids)
    if njb not in _prog_cache:
        _prog_cache[njb] = _build_fused(njb)
    nc = _prog_cache[njb]

    core_ids = list(range(N_CORES))
    res = run_bass_kernel_spmd(nc, maps, core_ids)

    global last_results
    last_results = (res,)
    acc = np.zeros((B, D), np.float32)
    for i in range(N_CORES):
        acc += res.results[i]["out"].astype(np.float32)
    return acc.astype(np.float16)[:, None, :]
